# revision 32
# baseline (speedup 1.0000x reference)
"""ActionEncoder Trainium2 kernel (8 NeuronCores, data-parallel over actions).

Strategy (v5 = v4 + per-gc idx loads):
  - Shard the 65536-row pick/trans/move action axes across 8 cores (8192 each).
  - L1 is linear before the activation: fold W1 into the tables on the host
    (T'_t = table @ W1_slice, b1 folded into the agv tables), so layer 1 is
    gathers + adds.  Tables are quantized to int8 with one GLOBAL scale s
    (all tables share sigma); s is folded into W2 on the host since
    lrelu(s*x) = s*lrelu(x).  Gather bytes halve vs bf16.
  - Plain (non-transposed) gathers: rows land on partitions.  DVE sums the
    gathered int8 table pairs (int8 in -> bf16 out), PE transposes via
    identity matmuls into f32 PSUM (pick's final pair-sum rides the PSUM
    accumulation), ACT applies lrelu PSUM -> SBUF aT (absorbs the copy).
  - PE: y^T = W2g^T @ a per 128-chunk with PSUM accumulation over the 4
    de-interleave groups; W2 rows host-permuted.  ACT copies PSUM -> bf16,
    DMA out.  wait rows are a host-side broadcast; b2 added on the host.

Steady state (from perfetto): ~19.2us per 1024-row gather-call group, all
four engine classes near their floors (ACT ~20us: 12 lrelu + 6 copies;
DVE ~18.4us: 8 int8 pair-add/converts; DMA engines ~17.6us: 8x32KB gather
packets + 6 output writes; PE ~17.5us: 128 transposes + 48 W2 matmuls).
Things measured and rejected (all regress or are neutral):
  - y-copies or adds on Pool: GPSIMD cannot access PSUM; Pool compute ops
    also head-of-line block descgen (sem waits + ucode mode switches)
    and collapse the gather stream (383us).
  - copies split ACT/DVE: perturbs the DVE gather-WAR pacer, higher
    variance (251-297us) than all-ACT (247.3-247.9us).
  - deeper g buffering (24/28): neutral; pipeline slack is capped by PSUM
    (tp+ps use all 8 banks) and the h pool, not g.
  - transposed gathers (v3): parallel-queue xbar corruption forces one
    queue, strided 2B writes are slow.
  - fp8 tables so PE eats gathers directly (skipping the DVE converts):
    e4m3 quant error ~3.6% > 2e-2 gate; e3m4 passes (~1.8%) but doubles
    transpose count (PE +7.2us/gc, new bottleneck); DoubleRow is e4/e5-only.
  - int16-bitcast 2x DVE adds: int8 pair sums need 9 bits (carries cross
    byte lanes); 7-bit tables would halve table precision (~2.1% > gate).
"""
import os
import sys

sys.path.insert(0, "/opt/trn_rl_repo")

import numpy as np
import ml_dtypes

P = 128
N2 = 512                # rows per compute supertile
NG = 1024               # rows per gather call (plain: 65 descs/DMA, fits ring)
ROWS = 8192             # rows per stream per core
NSUP = ROWS // N2       # 16 supertiles per stream
NCALL = ROWS // NG      # 8 gather calls per slot
GIDX = NG // 16         # idx columns per gather call (64)
NCORES = 8
D = 256                 # raw embedding dim per table
V_AGV, V_MACH, V_OP = 5000, 10000, 50000
V_OPC = 16384           # per-core compacted op table rows
H, O = 512, 256         # hidden / out dims
G = 4                   # de-interleave groups (cu, b)
SIDX = N2 // 16         # idx columns per supertile (64)
MC2 = O // P            # 2 out chunks

# gather slots: 0 pick_agv, 1 pick_opf, 2 pick_opt, 3 pick_mach,
#               4 trans_agv, 5 trans_mach, 6 move_agv, 7 move_mach

_NC_CACHE = {}


def _build_nc():
    import concourse.bass as bass  # noqa: F401
    from concourse import bacc
    import concourse.mybir as mybir
    from concourse.tile import TileContext

    bf16 = mybir.dt.bfloat16
    f32 = mybir.dt.float32
    i16 = mybir.dt.int16
    i8 = mybir.dt.int8

    nc = bacc.Bacc(num_swdge_queues=4)

    # int8 projected tables (h-space); global dequant scale folded into w2
    t_ap = nc.declare_dram_parameter("t_ap", [V_AGV, H], i8, isOutput=False)
    t_opf = nc.declare_dram_parameter("t_opf", [V_OPC, H], i8, isOutput=False)
    t_opt = nc.declare_dram_parameter("t_opt", [V_OPC, H], i8, isOutput=False)
    t_mp = nc.declare_dram_parameter("t_mp", [V_MACH, H], i8, isOutput=False)
    t_at = nc.declare_dram_parameter("t_at", [V_AGV, H], i8, isOutput=False)
    t_mt = nc.declare_dram_parameter("t_mt", [V_MACH, H], i8, isOutput=False)
    t_am = nc.declare_dram_parameter("t_am", [V_AGV, H], i8, isOutput=False)
    t_mm = nc.declare_dram_parameter("t_mm", [V_MACH, H], i8, isOutput=False)

    # gc-major so each gather-call group's indices load as one small DMA
    idx_all = nc.declare_dram_parameter(
        "idx_all", [NCALL, P, 8, GIDX], i16, isOutput=False)
    # w2 scaled: w2all[p, sid, kc, o] = s * W2_sid[kc*128 + p, o]
    w2all = nc.declare_dram_parameter("w2all", [P, 3, G, O], bf16, isOutput=False)
    ident = nc.declare_dram_parameter("ident", [P, P], bf16, isOutput=False)

    out = nc.declare_dram_parameter("out", [3, NSUP, P, MC2, N2], bf16, isOutput=True)

    JB = N2 // P  # 4 row blocks per supertile

    with TileContext(nc) as tc:
        with (
            tc.tile_pool(name="const", bufs=1) as const_tp,
            tc.tile_pool(name="g", bufs=20) as g_tp,
            tc.tile_pool(name="h", bufs=8) as h_tp,
            tc.tile_pool(name="a", bufs=6) as a_tp,
            tc.tile_pool(name="y", bufs=8) as y_tp,
            tc.tile_pool(name="tp", bufs=2, space="PSUM") as tp_tp,
            tc.tile_pool(name="ps", bufs=2, space="PSUM") as ps_tp,
        ):
            idx_sb = const_tp.tile([P, NCALL, 8, GIDX], dtype=i16)
            for gc in range(NCALL):
                nc.sync.dma_start(out=idx_sb[:, gc], in_=idx_all[gc])
            id_sb = const_tp.tile([P, P], dtype=bf16)
            nc.sync.dma_start(out=id_sb[:], in_=ident[:])
            w2_sb = const_tp.tile([P, 3, G, O], dtype=bf16)
            nc.sync.dma_start(out=w2_sb[:], in_=w2all[:])

            streams = [
                (0, [(t_ap, 0), (t_opf, 1), (t_opt, 2), (t_mp, 3)]),
                (1, [(t_at, 4), (t_mt, 5)]),
                (2, [(t_am, 6), (t_mm, 7)]),
            ]

            GJB = NG // P  # 8 row blocks per gather call
            slot_tiles = [{} for _ in range(NCALL)]

            def emit_gathers(gc):
                # one 1024-row gather per slot per group (desc-gen is serial
                # on the Pool engine: fewer, bigger calls)
                for sid, tables in streams:
                    for ti, (tab, slot) in enumerate(tables):
                        g = g_tp.tile([P, GJB, H], dtype=i8, tag="g")
                        nc.gpsimd.dma_gather(
                            out_ap=g[:, :, :], in_ap=tab[:],
                            idxs_ap=idx_sb[:, gc, slot, :],
                            num_idxs=NG, num_idxs_reg=NG,
                            elem_size=H, transpose=False,
                            queue_num=slot % 4,
                        )
                        slot_tiles[gc][slot] = g

            def emit_compute(gc):
                for sub in range(NG // N2):
                    for sid, tables in streams:
                        c = gc * (NG // N2) + sub
                        gsl = [slot_tiles[gc][slot][:, sub * JB:(sub + 1) * JB, :]
                               .rearrange("p j q -> p (j q)")
                               for _, slot in tables]

                        # sum int8 table pairs on DVE (int8 in -> bf16 out);
                        # pick's final pair-sum rides the PE transpose below
                        hs = []
                        if sid == 0:
                            h01 = h_tp.tile([P, JB, H], dtype=bf16, tag="h")
                            nc.vector.tensor_add(
                                out=h01[:].rearrange("p j q -> p (j q)"),
                                in0=gsl[0], in1=gsl[1])
                            h23 = h_tp.tile([P, JB, H], dtype=bf16, tag="h")
                            nc.vector.tensor_add(
                                out=h23[:].rearrange("p j q -> p (j q)"),
                                in0=gsl[2], in1=gsl[3])
                            hs = [h01, h23]
                        else:
                            h = h_tp.tile([P, JB, H], dtype=bf16, tag="h")
                            nc.vector.tensor_add(
                                out=h[:].rearrange("p j q -> p (j q)"),
                                in0=gsl[0], in1=gsl[1])
                            hs = [h]

                        # PE transpose-accumulate into f32 PSUM (sums the hs),
                        # ACT applies lrelu PSUM -> SBUF aT (absorbs the copy)
                        aT = a_tp.tile([P, G, N2], dtype=bf16, tag="a")
                        for hp in range(2):          # hc pairs
                            tp = tp_tp.tile([P, 2, N2], dtype=f32, tag="tp")
                            for hh in range(2):
                                hc = hp * 2 + hh
                                for j in range(JB):
                                    for hi, hx in enumerate(hs):
                                        nc.tensor.matmul(
                                            tp[:, hh, j * P:(j + 1) * P],
                                            lhsT=hx[:, j, hc * P:(hc + 1) * P],
                                            rhs=id_sb[:],
                                            start=(hi == 0),
                                            stop=(hi == len(hs) - 1),
                                        )
                            nc.scalar.activation(
                                out=aT[:, hp * 2:(hp + 1) * 2, :],
                                in_=tp[:],
                                func=mybir.ActivationFunctionType.Lrelu,
                                alpha=0.01)
                        # W2 into a 2-bank PSUM tile (one copy + one DMA out)
                        yps = ps_tp.tile([P, MC2, N2], dtype=f32, tag="yps")
                        for mc in range(MC2):
                            for kc in range(G):
                                nc.tensor.matmul(
                                    yps[:, mc, :],
                                    lhsT=w2_sb[:, sid, kc, mc * P:(mc + 1) * P],
                                    rhs=aT[:, kc, :],
                                    start=(kc == 0), stop=(kc == G - 1),
                                )
                        y_sb = y_tp.tile([P, MC2, N2], dtype=bf16, tag="ysb")
                        # PSUM->SBUF copy on ACT mid-pipe (DVE is the gather
                        # pacer; Pool cannot touch PSUM).  In the drain (last
                        # two supertiles) ACT is the straggler and DVE idles,
                        # so those copies go to DVE.
                        if c >= NSUP - 2:
                            nc.vector.tensor_copy(out=y_sb[:], in_=yps[:])
                        else:
                            nc.scalar.copy(out=y_sb[:], in_=yps[:])
                        nc.sync.dma_start(out=out[sid, c], in_=y_sb[:])

            # Pool runs ONLY descgen (mixing compute onto Pool serializes the
            # gather stream behind sem waits + ucode mode switches), so the
            # natural emission order keeps descgen ahead of compute.
            for gc in range(NCALL):
                emit_gathers(gc)
                emit_compute(gc)
    nc.compile()
    return nc


def _get_nc():
    if "nc" not in _NC_CACHE:
        _NC_CACHE["nc"] = _build_nc()
    return _NC_CACHE["nc"]


def _pack_idx(ix):
    """[ROWS] -> [NCALL, 128, GIDX] int16 dma_gather layout
    (idx i of call gc at partition i%16 (replicated x8), col i//16)."""
    a = ix.reshape(NCALL, GIDX, 16).transpose(0, 2, 1)       # [gc, 16, GIDX]
    a = np.tile(a, (1, 8, 1))                                # [gc, 128, GIDX]
    return a.astype(np.int16)


def kernel(**inputs):
    inp = {k: np.asarray(v) for k, v in inputs.items()}
    n_wait = int(inp["n_wait"])

    bf = ml_dtypes.bfloat16
    agv = inp["agv_emb"].astype(np.float32)
    mach = inp["machine_emb"].astype(np.float32)
    op = inp["operation_emb"].astype(np.float32)

    w1p = inp["pick_w1"].astype(np.float32)
    w1t = inp["trans_w1"].astype(np.float32)
    w1m = inp["move_w1"].astype(np.float32)
    b1p = inp["pick_b1"].astype(np.float32)
    b1t = inp["trans_b1"].astype(np.float32)
    b1m = inp["move_b1"].astype(np.float32)

    # projected tables in f32 (b1 folded into the agv ones)
    t_ap = agv @ w1p[0:D] + b1p
    opf_full = op @ w1p[D:2 * D]
    opt_full = op @ w1p[2 * D:3 * D]
    t_mp = mach @ w1p[3 * D:4 * D]
    t_at = agv @ w1t[0:D] + b1t
    t_mt = mach @ w1t[D:2 * D]
    t_am = agv @ w1m[0:D] + b1m
    t_mm = mach @ w1m[D:2 * D]

    # global int8 scale (all tables share sigma; clip at 4.5 sigma)
    sig = float(np.sqrt(np.mean([np.var(t_ap), np.var(t_mp), np.var(t_mt),
                                 np.var(t_mm), np.var(opf_full[:4096]),
                                 np.var(opt_full[:4096])])))
    amax = max(float(np.max(np.abs(t))) for t in
               (t_ap, t_mp, t_at, t_mt, t_am, t_mm, opf_full, opt_full))
    clip = min(amax, 4.5 * sig)
    s = clip / 127.0

    def q8(t):
        return np.clip(np.round(t / s), -127, 127).astype(np.int8)

    # w2 scaled: w2all[p, sid, kc, o] = s * W2_sid[kc*128 + p, o]
    hperm = np.arange(H)
    w2all = np.empty((P, 3, G, O), np.float32)
    for sid, pre in enumerate(("pick", "trans", "move")):
        w2 = inp[f"{pre}_w2"].astype(np.float32) * s     # [H, O]
        w2p = w2[hperm].reshape(G, P, O)                 # [g, p, o]
        w2all[:, sid] = w2p.transpose(1, 0, 2)
    w2all = w2all.astype(bf)

    b2 = np.stack([inp[f"{pre}_b2"].astype(np.float32)
                   for pre in ("pick", "trans", "move")], 0)  # [3, O]

    idx = {k: inp[k].astype(np.int64) for k in (
        "pick_agv", "pick_op_from", "pick_op_to", "pick_machine",
        "trans_agv", "trans_machine", "move_agv", "move_machine")}

    shared = {
        "t_ap": q8(t_ap), "t_mp": q8(t_mp), "t_at": q8(t_at), "t_mt": q8(t_mt),
        "t_am": q8(t_am), "t_mm": q8(t_mm),
        "w2all": w2all,
        "ident": np.eye(P, dtype=np.float32).astype(bf),
    }
    opf8_full = q8(opf_full)
    opt8_full = q8(opt_full)

    in_maps = []
    for c in range(NCORES):
        sl = slice(c * ROWS, (c + 1) * ROWS)
        opf = idx["pick_op_from"][sl]
        opt = idx["pick_op_to"][sl]
        uniq = np.unique(np.concatenate([opf, opt]))
        t_opf = np.zeros((V_OPC, H), np.int8)
        t_opf[: uniq.size] = opf8_full[uniq]
        t_opt = np.zeros((V_OPC, H), np.int8)
        t_opt[: uniq.size] = opt8_full[uniq]
        opf_r = np.searchsorted(uniq, opf)
        opt_r = np.searchsorted(uniq, opt)

        blocks = [
            _pack_idx(idx["pick_agv"][sl]),
            _pack_idx(opf_r),
            _pack_idx(opt_r),
            _pack_idx(idx["pick_machine"][sl]),
            _pack_idx(idx["trans_agv"][sl]),
            _pack_idx(idx["trans_machine"][sl]),
            _pack_idx(idx["move_agv"][sl]),
            _pack_idx(idx["move_machine"][sl]),
        ]
        # [slot, gc, 128, GIDX] -> [gc, 128, slot, GIDX]
        idx_arr = np.stack(blocks, 0).transpose(1, 2, 0, 3)

        in_maps.append({
            **shared,
            "t_opf": t_opf, "t_opt": t_opt,
            "idx_all": np.ascontiguousarray(idx_arr),
        })

    trace = bool(os.environ.get("ACTION_ENC_TRACE"))
    if trace:
        _install_trace_shim()
    from concourse.bass_utils import run_bass_kernel_spmd

    nc = _get_nc()
    res = run_bass_kernel_spmd(
        nc, in_maps, core_ids=list(range(NCORES)), trace=trace,
    )
    if trace:
        print(f"HW exec time: {res.exec_time_ns} ns")

    # reassemble: out [3, NSUP, P, MC2, N2] bf16 per core
    outs = np.stack([np.asarray(res.results[c]["out"]) for c in range(NCORES)], 0)
    outs = outs.astype(np.float32)
    # y[core, s, row, feat]: row = c*N2 + i, feat = mc*128 + p
    y = outs.transpose(0, 1, 2, 5, 4, 3).reshape(NCORES, 3, ROWS, O)
    y = y + b2[None, :, None, :]

    wait_out = np.broadcast_to(inp["wait_emb"].astype(np.float32), (n_wait, O))
    pick_out = y[:, 0].reshape(NCORES * ROWS, O)
    trans_out = y[:, 1].reshape(NCORES * ROWS, O)
    move_out = y[:, 2].reshape(NCORES * ROWS, O)
    return np.concatenate([wait_out, pick_out, trans_out, move_out], 0)


def _install_trace_shim():
    import types
    try:
        import antenv.axon_hooks  # noqa: F401
    except ImportError:
        from trn_agent_boot.trn_boot import _ntff_profile_via_ctypes
        import antenv
        hook = _ntff_profile_via_ctypes("/opt/axon/libaxon_pjrt.so")
        mod = types.ModuleType("antenv.axon_hooks")
        mod.get_axon_ntff_profile_hook = lambda: hook
        mod.set_axon_ntff_profile_hook = lambda h: None
        sys.modules["antenv.axon_hooks"] = mod
        antenv.axon_hooks = mod
    import concourse.bass_utils as bum
    bum.upload_artifacts = lambda tmpdir: f"local:{tmpdir}"


# revision 33
# speedup vs baseline: 1.2059x; 1.2059x over previous
"""ActionEncoder Trainium2 kernel (8 NeuronCores, data-parallel over actions).

Strategy (v5 = v4 + per-gc idx loads):
  - Shard the 65536-row pick/trans/move action axes across 8 cores (8192 each).
  - L1 is linear before the activation: fold W1 into the tables on the host
    (T'_t = table @ W1_slice, b1 folded into the agv tables), so layer 1 is
    gathers + adds.  Tables are quantized to int8 with one GLOBAL scale s
    (all tables share sigma); s is folded into W2 on the host since
    lrelu(s*x) = s*lrelu(x).  Gather bytes halve vs bf16.
  - Plain (non-transposed) gathers: rows land on partitions.  DVE sums the
    gathered int8 table pairs (int8 in -> bf16 out), PE transposes via
    identity matmuls into f32 PSUM (pick's final pair-sum rides the PSUM
    accumulation), ACT applies lrelu PSUM -> SBUF aT (absorbs the copy).
  - PE: y^T = W2g^T @ a per 128-chunk with PSUM accumulation over the 4
    de-interleave groups; W2 rows host-permuted.  ACT copies PSUM -> bf16,
    DMA out.  wait rows are a host-side broadcast; b2 added on the host.

Steady state (from perfetto): ~19.2us per 1024-row gather-call group, all
four engine classes near their floors (ACT ~20us: 12 lrelu + 6 copies;
DVE ~18.4us: 8 int8 pair-add/converts; DMA engines ~17.6us: 8x32KB gather
packets + 6 output writes; PE ~17.5us: 128 transposes + 48 W2 matmuls).
Things measured and rejected (all regress or are neutral):
  - y-copies or adds on Pool: GPSIMD cannot access PSUM; Pool compute ops
    also head-of-line block descgen (sem waits + ucode mode switches)
    and collapse the gather stream (383us).
  - copies split ACT/DVE: perturbs the DVE gather-WAR pacer, higher
    variance (251-297us) than all-ACT (247.3-247.9us).
  - deeper g buffering (24/28): neutral; pipeline slack is capped by PSUM
    (tp+ps use all 8 banks) and the h pool, not g.
  - transposed gathers (v3): parallel-queue xbar corruption forces one
    queue, strided 2B writes are slow.
  - fp8 tables so PE eats gathers directly (skipping the DVE converts):
    e4m3 quant error ~3.6% > 2e-2 gate; e3m4 passes (~1.8%) but doubles
    transpose count (PE +7.2us/gc, new bottleneck); DoubleRow is e4/e5-only.
  - int16-bitcast 2x DVE adds: int8 pair sums need 9 bits (carries cross
    byte lanes); 7-bit tables would halve table precision (~2.1% > gate).
"""
import os
import sys

sys.path.insert(0, "/opt/trn_rl_repo")

import numpy as np
import ml_dtypes

P = 128
N2 = 512                # rows per compute supertile
NG = 1024               # rows per gather call (plain: 65 descs/DMA, fits ring)
ROWS = 8192             # rows per stream per core
NSUP = ROWS // N2       # 16 supertiles per stream
NCALL = ROWS // NG      # 8 gather calls per slot
GIDX = NG // 16         # idx columns per gather call (64)
NCORES = 8
D = 256                 # raw embedding dim per table
V_AGV, V_MACH, V_OP = 5000, 10000, 50000
V_OPC = 16384           # per-core compacted op table rows
H, O = 512, 256         # hidden / out dims
G = 4                   # de-interleave groups (cu, b)
SIDX = N2 // 16         # idx columns per supertile (64)
MC2 = O // P            # 2 out chunks

# gather slots: 0 pick_agv, 1 pick_opf, 2 pick_opt, 3 pick_mach,
#               4 trans_agv, 5 trans_mach, 6 move_agv, 7 move_mach

_NC_CACHE = {}


def _build_nc():
    import concourse.bass as bass  # noqa: F401
    from concourse import bacc
    import concourse.mybir as mybir
    from concourse.tile import TileContext

    bf16 = mybir.dt.bfloat16
    f32 = mybir.dt.float32
    i16 = mybir.dt.int16
    i8 = mybir.dt.int8

    nc = bacc.Bacc(num_swdge_queues=4)

    # int8 projected tables (h-space); global dequant scale folded into w2
    t_ap = nc.declare_dram_parameter("t_ap", [V_AGV, H], i8, isOutput=False)
    t_opf = nc.declare_dram_parameter("t_opf", [V_OPC, H], i8, isOutput=False)
    t_opt = nc.declare_dram_parameter("t_opt", [V_OPC, H], i8, isOutput=False)
    t_mp = nc.declare_dram_parameter("t_mp", [V_MACH, H], i8, isOutput=False)
    t_at = nc.declare_dram_parameter("t_at", [V_AGV, H], i8, isOutput=False)
    t_mt = nc.declare_dram_parameter("t_mt", [V_MACH, H], i8, isOutput=False)
    t_am = nc.declare_dram_parameter("t_am", [V_AGV, H], i8, isOutput=False)
    t_mm = nc.declare_dram_parameter("t_mm", [V_MACH, H], i8, isOutput=False)

    # gc-major so each gather-call group's indices load as one small DMA
    idx_all = nc.declare_dram_parameter(
        "idx_all", [NCALL, P, 8, GIDX], i16, isOutput=False)
    # w2 scaled: w2all[p, sid, kc, o] = s * W2_sid[kc*128 + p, o]
    w2all = nc.declare_dram_parameter("w2all", [P, 3, G, O], bf16, isOutput=False)
    ident = nc.declare_dram_parameter("ident", [P, P], bf16, isOutput=False)

    out = nc.declare_dram_parameter("out", [3, NSUP, P, MC2, N2], bf16, isOutput=True)

    JB = N2 // P  # 4 row blocks per supertile

    with TileContext(nc) as tc:
        with (
            tc.tile_pool(name="const", bufs=1) as const_tp,
            tc.tile_pool(name="g", bufs=20) as g_tp,
            tc.tile_pool(name="h", bufs=8) as h_tp,
            tc.tile_pool(name="a", bufs=6) as a_tp,
            tc.tile_pool(name="y", bufs=8) as y_tp,
            tc.tile_pool(name="tp", bufs=2, space="PSUM") as tp_tp,
            tc.tile_pool(name="ps", bufs=2, space="PSUM") as ps_tp,
        ):
            idx_sb = const_tp.tile([P, NCALL, 8, GIDX], dtype=i16)
            for gc in range(NCALL):
                nc.sync.dma_start(out=idx_sb[:, gc], in_=idx_all[gc])
            id_sb = const_tp.tile([P, P], dtype=bf16)
            nc.sync.dma_start(out=id_sb[:], in_=ident[:])
            w2_sb = const_tp.tile([P, 3, G, O], dtype=bf16)
            nc.sync.dma_start(out=w2_sb[:], in_=w2all[:])

            streams = [
                (0, [(t_ap, 0), (t_opf, 1), (t_opt, 2), (t_mp, 3)]),
                (1, [(t_at, 4), (t_mt, 5)]),
                (2, [(t_am, 6), (t_mm, 7)]),
            ]

            GJB = NG // P  # 8 row blocks per gather call
            slot_tiles = [{} for _ in range(NCALL)]

            def emit_gathers(gc):
                # one 1024-row gather per slot per group (desc-gen is serial
                # on the Pool engine: fewer, bigger calls)
                for sid, tables in streams:
                    for ti, (tab, slot) in enumerate(tables):
                        g = g_tp.tile([P, GJB, H], dtype=i8, tag="g")
                        nc.gpsimd.dma_gather(
                            out_ap=g[:, :, :], in_ap=tab[:],
                            idxs_ap=idx_sb[:, gc, slot, :],
                            num_idxs=NG, num_idxs_reg=NG,
                            elem_size=H, transpose=False,
                            queue_num=slot % 4,
                        )
                        slot_tiles[gc][slot] = g

            def emit_compute(gc):
                for sub in range(NG // N2):
                    for sid, tables in streams:
                        c = gc * (NG // N2) + sub
                        gsl = [slot_tiles[gc][slot][:, sub * JB:(sub + 1) * JB, :]
                               .rearrange("p j q -> p (j q)")
                               for _, slot in tables]

                        # sum int8 table pairs on DVE (int8 in -> bf16 out);
                        # pick's final pair-sum rides the PE transpose below
                        hs = []
                        if sid == 0:
                            h01 = h_tp.tile([P, JB, H], dtype=bf16, tag="h")
                            nc.vector.tensor_add(
                                out=h01[:].rearrange("p j q -> p (j q)"),
                                in0=gsl[0], in1=gsl[1])
                            h23 = h_tp.tile([P, JB, H], dtype=bf16, tag="h")
                            nc.vector.tensor_add(
                                out=h23[:].rearrange("p j q -> p (j q)"),
                                in0=gsl[2], in1=gsl[3])
                            hs = [h01, h23]
                        else:
                            h = h_tp.tile([P, JB, H], dtype=bf16, tag="h")
                            nc.vector.tensor_add(
                                out=h[:].rearrange("p j q -> p (j q)"),
                                in0=gsl[0], in1=gsl[1])
                            hs = [h]

                        # PE transpose-accumulate into f32 PSUM (sums the hs),
                        # ACT applies lrelu PSUM -> SBUF aT (absorbs the copy)
                        aT = a_tp.tile([P, G, N2], dtype=bf16, tag="a")
                        for hp in range(2):          # hc pairs
                            tp = tp_tp.tile([P, 2, N2], dtype=f32, tag="tp")
                            for hh in range(2):
                                hc = hp * 2 + hh
                                for j in range(JB):
                                    for hi, hx in enumerate(hs):
                                        nc.tensor.matmul(
                                            tp[:, hh, j * P:(j + 1) * P],
                                            lhsT=hx[:, j, hc * P:(hc + 1) * P],
                                            rhs=id_sb[:],
                                            start=(hi == 0),
                                            stop=(hi == len(hs) - 1),
                                        )
                            nc.scalar.activation(
                                out=aT[:, hp * 2:(hp + 1) * 2, :],
                                in_=tp[:],
                                func=mybir.ActivationFunctionType.Lrelu,
                                alpha=0.01)
                        # W2 into a 2-bank PSUM tile (one copy + one DMA out)
                        yps = ps_tp.tile([P, MC2, N2], dtype=f32, tag="yps")
                        for mc in range(MC2):
                            for kc in range(G):
                                nc.tensor.matmul(
                                    yps[:, mc, :],
                                    lhsT=w2_sb[:, sid, kc, mc * P:(mc + 1) * P],
                                    rhs=aT[:, kc, :],
                                    start=(kc == 0), stop=(kc == G - 1),
                                )
                        y_sb = y_tp.tile([P, MC2, N2], dtype=bf16, tag="ysb")
                        # PSUM->SBUF copy on ACT: DVE is the gather-stream
                        # pacer (WAR waits target DVE progress), so keep DVE
                        # free of extra work; Pool cannot touch PSUM.
                        nc.scalar.copy(out=y_sb[:], in_=yps[:])
                        nc.sync.dma_start(out=out[sid, c], in_=y_sb[:])

            # Pool runs ONLY descgen (mixing compute onto Pool serializes the
            # gather stream behind sem waits + ucode mode switches), so the
            # natural emission order keeps descgen ahead of compute.
            for gc in range(NCALL):
                emit_gathers(gc)
                emit_compute(gc)
    nc.compile()
    return nc


def _get_nc():
    if "nc" not in _NC_CACHE:
        _NC_CACHE["nc"] = _build_nc()
    return _NC_CACHE["nc"]


def _pack_idx(ix):
    """[ROWS] -> [NCALL, 128, GIDX] int16 dma_gather layout
    (idx i of call gc at partition i%16 (replicated x8), col i//16)."""
    a = ix.reshape(NCALL, GIDX, 16).transpose(0, 2, 1)       # [gc, 16, GIDX]
    a = np.tile(a, (1, 8, 1))                                # [gc, 128, GIDX]
    return a.astype(np.int16)


def kernel(**inputs):
    inp = {k: np.asarray(v) for k, v in inputs.items()}
    n_wait = int(inp["n_wait"])

    bf = ml_dtypes.bfloat16
    agv = inp["agv_emb"].astype(np.float32)
    mach = inp["machine_emb"].astype(np.float32)
    op = inp["operation_emb"].astype(np.float32)

    w1p = inp["pick_w1"].astype(np.float32)
    w1t = inp["trans_w1"].astype(np.float32)
    w1m = inp["move_w1"].astype(np.float32)
    b1p = inp["pick_b1"].astype(np.float32)
    b1t = inp["trans_b1"].astype(np.float32)
    b1m = inp["move_b1"].astype(np.float32)

    # projected tables in f32 (b1 folded into the agv ones)
    t_ap = agv @ w1p[0:D] + b1p
    opf_full = op @ w1p[D:2 * D]
    opt_full = op @ w1p[2 * D:3 * D]
    t_mp = mach @ w1p[3 * D:4 * D]
    t_at = agv @ w1t[0:D] + b1t
    t_mt = mach @ w1t[D:2 * D]
    t_am = agv @ w1m[0:D] + b1m
    t_mm = mach @ w1m[D:2 * D]

    # global int8 scale (all tables share sigma; clip at 4.5 sigma)
    sig = float(np.sqrt(np.mean([np.var(t_ap), np.var(t_mp), np.var(t_mt),
                                 np.var(t_mm), np.var(opf_full[:4096]),
                                 np.var(opt_full[:4096])])))
    amax = max(float(np.max(np.abs(t))) for t in
               (t_ap, t_mp, t_at, t_mt, t_am, t_mm, opf_full, opt_full))
    clip = min(amax, 4.5 * sig)
    s = clip / 127.0

    def q8(t):
        return np.clip(np.round(t / s), -127, 127).astype(np.int8)

    # w2 scaled: w2all[p, sid, kc, o] = s * W2_sid[kc*128 + p, o]
    hperm = np.arange(H)
    w2all = np.empty((P, 3, G, O), np.float32)
    for sid, pre in enumerate(("pick", "trans", "move")):
        w2 = inp[f"{pre}_w2"].astype(np.float32) * s     # [H, O]
        w2p = w2[hperm].reshape(G, P, O)                 # [g, p, o]
        w2all[:, sid] = w2p.transpose(1, 0, 2)
    w2all = w2all.astype(bf)

    b2 = np.stack([inp[f"{pre}_b2"].astype(np.float32)
                   for pre in ("pick", "trans", "move")], 0)  # [3, O]

    idx = {k: inp[k].astype(np.int64) for k in (
        "pick_agv", "pick_op_from", "pick_op_to", "pick_machine",
        "trans_agv", "trans_machine", "move_agv", "move_machine")}

    shared = {
        "t_ap": q8(t_ap), "t_mp": q8(t_mp), "t_at": q8(t_at), "t_mt": q8(t_mt),
        "t_am": q8(t_am), "t_mm": q8(t_mm),
        "w2all": w2all,
        "ident": np.eye(P, dtype=np.float32).astype(bf),
    }
    opf8_full = q8(opf_full)
    opt8_full = q8(opt_full)

    in_maps = []
    for c in range(NCORES):
        sl = slice(c * ROWS, (c + 1) * ROWS)
        opf = idx["pick_op_from"][sl]
        opt = idx["pick_op_to"][sl]
        uniq = np.unique(np.concatenate([opf, opt]))
        t_opf = np.zeros((V_OPC, H), np.int8)
        t_opf[: uniq.size] = opf8_full[uniq]
        t_opt = np.zeros((V_OPC, H), np.int8)
        t_opt[: uniq.size] = opt8_full[uniq]
        opf_r = np.searchsorted(uniq, opf)
        opt_r = np.searchsorted(uniq, opt)

        blocks = [
            _pack_idx(idx["pick_agv"][sl]),
            _pack_idx(opf_r),
            _pack_idx(opt_r),
            _pack_idx(idx["pick_machine"][sl]),
            _pack_idx(idx["trans_agv"][sl]),
            _pack_idx(idx["trans_machine"][sl]),
            _pack_idx(idx["move_agv"][sl]),
            _pack_idx(idx["move_machine"][sl]),
        ]
        # [slot, gc, 128, GIDX] -> [gc, 128, slot, GIDX]
        idx_arr = np.stack(blocks, 0).transpose(1, 2, 0, 3)

        in_maps.append({
            **shared,
            "t_opf": t_opf, "t_opt": t_opt,
            "idx_all": np.ascontiguousarray(idx_arr),
        })

    trace = bool(os.environ.get("ACTION_ENC_TRACE"))
    if trace:
        _install_trace_shim()
    from concourse.bass_utils import run_bass_kernel_spmd

    nc = _get_nc()
    res = run_bass_kernel_spmd(
        nc, in_maps, core_ids=list(range(NCORES)), trace=trace,
    )
    if trace:
        print(f"HW exec time: {res.exec_time_ns} ns")

    # reassemble: out [3, NSUP, P, MC2, N2] bf16 per core
    outs = np.stack([np.asarray(res.results[c]["out"]) for c in range(NCORES)], 0)
    outs = outs.astype(np.float32)
    # y[core, s, row, feat]: row = c*N2 + i, feat = mc*128 + p
    y = outs.transpose(0, 1, 2, 5, 4, 3).reshape(NCORES, 3, ROWS, O)
    y = y + b2[None, :, None, :]

    wait_out = np.broadcast_to(inp["wait_emb"].astype(np.float32), (n_wait, O))
    pick_out = y[:, 0].reshape(NCORES * ROWS, O)
    trans_out = y[:, 1].reshape(NCORES * ROWS, O)
    move_out = y[:, 2].reshape(NCORES * ROWS, O)
    return np.concatenate([wait_out, pick_out, trans_out, move_out], 0)


def _install_trace_shim():
    import types
    try:
        import antenv.axon_hooks  # noqa: F401
    except ImportError:
        from trn_agent_boot.trn_boot import _ntff_profile_via_ctypes
        import antenv
        hook = _ntff_profile_via_ctypes("/opt/axon/libaxon_pjrt.so")
        mod = types.ModuleType("antenv.axon_hooks")
        mod.get_axon_ntff_profile_hook = lambda: hook
        mod.set_axon_ntff_profile_hook = lambda h: None
        sys.modules["antenv.axon_hooks"] = mod
        antenv.axon_hooks = mod
    import concourse.bass_utils as bum
    bum.upload_artifacts = lambda tmpdir: f"local:{tmpdir}"
